# revision 10
# baseline (speedup 1.0000x reference)
"""BasicMPNN Trainium2 kernel (8 NeuronCores, SPMD).

Math: since the message MLP is linear and applied before segment_sum,
    m = concat([h[send], h[rec], e], 1) @ msg_W + msg_b
    agg = segment_sum(m, rec)
        = (A @ h) @ W1 + (deg * h) @ W2 + s_attr x u_l + deg x c_l
  where A[r, s] = multiplicity of edge s->r, deg = A @ 1,
  s_attr[r] = sum of edge_attr into r, u_l = W_edge @ W3_l,
  c_l = b_edge @ W3_l + msg_b_l.
So each layer needs ONE sparse gather+reduce (A @ h) plus small dense
matmuls. Everything is computed feature-major on-chip ([64, nodes]).

Sharding: receiver-range partition, 12500 nodes / core. Full h lives
replicated in each core's DRAM (rebuilt per layer by AllGather). The
SpMM gathers h[send] rows with dma_gather (int16 idx -> 4 row-stripe
subtables of 25000 rows), then a one-hot matmul (S^T built on DVE by
comparing iota to the per-edge local receiver id) accumulates each
128-receiver window in PSUM, output directly feature-major.
"""

import os
import numpy as np

import concourse.bass as bass
import concourse.bacc as bacc
import concourse.tile as tile
from concourse import bass_utils, mybir, library_config
from concourse.masks import make_identity

dt = mybir.dt

# problem constants (hardcoded per contract)
N = 100000
E = 1600000
FIN = 16
H = 64
L = 3
NG = 64
N_CORES = 8
NC_N = N // N_CORES            # 12500 nodes per core
WIN = 128                      # receiver window
W = (NC_N + WIN - 1) // WIN    # 98 windows per core
NPAD = W * WIN                 # 12544
S = 4                          # send stripes (int16 idx limit)
VS = N // S                    # 25000 rows per stripe
SUPW = 4                       # windows per gather call group
DENSE_CHUNK = 512
PHASE = int(os.environ.get("K_PHASE", "4"))  # 1 embed+pool, 2 +AG, 3 +spmm, 4 full

_cache = {}


def _prep_structure(send, rec):
    """Host: bucket edges by (core, window, stripe); common (cross-core)
    tile counts; per-core idx / rec_local streams in global tile order."""
    core = rec // NC_N
    counts = np.zeros((N_CORES, W, S), np.int64)
    per_core = []
    for c in range(N_CORES):
        sel = core == c
        cs, cr = send[sel], rec[sel] - c * NC_N
        w = cr // WIN
        st = cs // VS
        key = (w * S + st).astype(np.int64)
        order = np.argsort(key, kind="stable")
        cs, cr, w, st, key = cs[order], cr[order], w[order], st[order], key[order]
        counts[c] = np.bincount(key, minlength=W * S).reshape(W, S)
        per_core.append((cs, cr, key))
    tiles = (np.max(counts, axis=0) + WIN - 1) // WIN  # [W, S] common
    # supergroups of SUPW windows
    sups = [list(range(i, min(i + SUPW, W))) for i in range(0, W, SUPW)]
    # global tile order: per sup, per stripe, per window
    tile_order = []          # list of (w, s)
    call_spec = []           # per (sup, s): (n_tiles,)
    win_tiles = [[] for _ in range(W)]   # w -> list of (call_idx, col_in_call)
    for sup in sups:
        for s in range(S):
            ntil = 0
            ci = len(call_spec)
            for w in sup:
                for _ in range(tiles[w, s]):
                    win_tiles[w].append((ci, ntil, len(tile_order)))
                    tile_order.append((w, s))
                    ntil += 1
            call_spec.append(ntil)
    TT = len(tile_order)
    return {
        "tiles": tiles, "sups": sups, "tile_order": tile_order,
        "call_spec": call_spec, "win_tiles": win_tiles, "TT": TT,
        "per_core": per_core,
    }


def _prep_core_arrays(structure, c):
    """Per-core idx (wrapped int16) and rec_local (f32) streams."""
    tiles = structure["tiles"]
    sups = structure["sups"]
    cs, cr, key = structure["per_core"][c]
    TT = structure["TT"]
    # bucket start offsets in the (w,s)-sorted edge array
    bc = np.bincount(key, minlength=W * S).reshape(W, S)
    starts = np.zeros(W * S + 1, np.int64)
    np.cumsum(bc.reshape(-1), out=starts[1:])
    idx_vals = np.zeros(TT * WIN, np.int16)
    rec_vals = np.full((TT * WIN,), -1.0, np.float32)
    t = 0
    for sup in sups:
        for s in range(S):
            for w in sup:
                k = w * S + s
                lo, n = starts[k], bc[w, s]
                nt = tiles[w, s]
                seg_idx = (cs[lo:lo + n] - s * VS).astype(np.int16)
                seg_rec = (cr[lo:lo + n] - w * WIN).astype(np.float32)
                idx_vals[t * WIN: t * WIN + n] = seg_idx
                rec_vals[t * WIN: t * WIN + n] = seg_rec
                t += nt
    assert t == TT
    # rec_local stream: [128, TT], column t = edges of tile t
    recl = rec_vals.reshape(TT, WIN).T.copy()
    # idx stream: per call, wrapped [16, n/16] replicated x8 -> concat cols
    blocks = []
    base = 0
    for ntil in structure["call_spec"]:
        n = ntil * WIN
        flat = idx_vals[base * WIN: base * WIN + n]
        wrapped = flat.reshape(n // 16, 16).T          # [16, n/16]
        blocks.append(np.tile(wrapped, (8, 1)))        # [128, n/16]
        base += ntil
    idx_sb = np.concatenate(blocks, axis=1)
    return idx_sb, recl


def _build_bass(structure):
    tiles = structure["tiles"]
    sups = structure["sups"]
    call_spec = structure["call_spec"]
    win_tiles = structure["win_tiles"]
    TT = structure["TT"]
    IDXCOLS = sum(ntil * WIN // 16 for ntil in call_spec)
    TMAXCALL = max(call_spec)

    nc = bacc.Bacc("TRN2", target_bir_lowering=False, debug=False,
                   num_devices=N_CORES)
    f32 = dt.float32

    # I/O
    xT_in = nc.dram_tensor("xT", [FIN + 1, NPAD], f32, kind="ExternalInput")
    degt_in = nc.dram_tensor("degt", [H, NPAD], f32, kind="ExternalInput")
    aggrows_in = nc.dram_tensor("aggrows", [2, NPAD], f32, kind="ExternalInput")
    mpool_in = nc.dram_tensor("mpool", [NPAD, NG], f32, kind="ExternalInput")
    idx_in = nc.dram_tensor("idx", [128, IDXCOLS], dt.int16, kind="ExternalInput")
    recl_in = nc.dram_tensor("recl", [128, TT], f32, kind="ExternalInput")
    w1e_in = nc.dram_tensor("w1e", [H + 2, L * H], f32, kind="ExternalInput")
    w2_in = nc.dram_tensor("w2", [H, L * H], f32, kind="ExternalInput")
    wu1e_in = nc.dram_tensor("wu1e", [H + 1, L * H], f32, kind="ExternalInput")
    wu2_in = nc.dram_tensor("wu2", [H, L * H], f32, kind="ExternalInput")
    wemb_in = nc.dram_tensor("wemb", [FIN + 1, H], f32, kind="ExternalInput")
    pool_out = nc.dram_tensor("pool_out", [NG, H], f32, kind="ExternalOutput")

    nchunks = [DENSE_CHUNK] * (NPAD // DENSE_CHUNK)
    if NPAD % DENSE_CHUNK:
        nchunks.append(NPAD % DENSE_CHUNK)

    with tile.TileContext(nc) as tc:
        with (
            tc.tile_pool(name="dram", bufs=1, space="DRAM") as dpool,
            tc.tile_pool(name="const", bufs=1) as cpool,
            tc.tile_pool(name="gpool", bufs=6) as gpool,
            tc.tile_pool(name="stp", bufs=4) as stpool,
            tc.tile_pool(name="dense", bufs=2) as dpool_sb,
            tc.tile_pool(name="stg", bufs=3) as stgpool,
            tc.tile_pool(name="ps_w", bufs=2, space="PSUM") as ps_w,
            tc.tile_pool(name="ps_tp", bufs=2, space="PSUM") as ps_tp,
            tc.tile_pool(name="ps_mm", bufs=3, space="PSUM") as ps_mm,
            tc.tile_pool(name="ps_pool", bufs=1, space="PSUM") as ps_pool,
        ):
            tables = [dpool.tile([N, H], f32, addr_space="Shared",
                                 name=f"table{l}") for l in range(L)]
            hslice = dpool.tile([NPAD, H], f32)

            nc.gpsimd.load_library(library_config.mlp)

            # ---- constants ----
            iota = cpool.tile([128, 128], f32)
            nc.gpsimd.iota(iota[:], pattern=[[1, 128]], base=0,
                           channel_multiplier=0,
                           allow_small_or_imprecise_dtypes=True)
            ident = cpool.tile([128, 128], f32)
            make_identity(nc, ident[:])
            recl = cpool.tile([128, TT], f32)
            nc.sync.dma_start(out=recl[:], in_=recl_in[:])
            w1e = cpool.tile([H + 2, L * H], f32)
            nc.sync.dma_start(out=w1e[:], in_=w1e_in[:])
            w2 = cpool.tile([H, L * H], f32)
            nc.sync.dma_start(out=w2[:], in_=w2_in[:])
            wu1e = cpool.tile([H + 1, L * H], f32)
            nc.sync.dma_start(out=wu1e[:], in_=wu1e_in[:])
            wu2 = cpool.tile([H, L * H], f32)
            nc.sync.dma_start(out=wu2[:], in_=wu2_in[:])
            wemb = cpool.tile([FIN + 1, H], f32)
            nc.sync.dma_start(out=wemb[:], in_=wemb_in[:])

            Ht = cpool.tile([H + 1, NPAD], f32)       # row H = ones
            nc.vector.memset(Ht[H:H + 1, :], 1.0)
            AggT = cpool.tile([H + 2, NPAD], f32)     # rows H,H+1 = s_attr,deg
            nc.sync.dma_start(out=AggT[H:H + 2, :], in_=aggrows_in[:])

            # ---- embed: Ht = (x @ W_embed + b_embed)^T ----
            col = 0
            for cw in nchunks:
                xt = dpool_sb.tile([FIN + 1, DENSE_CHUNK], f32, tag="xt")
                nc.sync.dma_start(out=xt[:, :cw], in_=xT_in[:, col:col + cw])
                ps = ps_mm.tile([H, DENSE_CHUNK], f32, tag="mm")
                nc.tensor.matmul(ps[:, :cw], wemb[:], xt[:, :cw],
                                 start=True, stop=True)
                nc.scalar.copy(out=Ht[0:H, col:col + cw], in_=ps[:, :cw])
                col += cw

            for l in range(L if PHASE >= 2 else 0):
                # ---- write back h -> hslice -> AllGather -> table ----
                for gi, sup in enumerate(sups):
                    stg = stgpool.tile([128, SUPW * H], f32, tag="stg")
                    for j, w in enumerate(sup):
                        tp = ps_tp.tile([128, H], f32, tag="tp")
                        nc.tensor.transpose(
                            out=tp[:], in_=Ht[0:H, w * WIN:(w + 1) * WIN],
                            identity=ident[0:H, 0:H])
                        nc.vector.tensor_copy(
                            out=stg[:, j * H:(j + 1) * H], in_=tp[:])
                    nw = len(sup)
                    dst = hslice[:].rearrange("(w p) f -> p w f", p=WIN)
                    nc.sync.dma_start(
                        out=dst[:, sup[0]:sup[0] + nw, :],
                        in_=stg[:, :nw * H].rearrange("p (w f) -> p w f", f=H))
                nc.gpsimd.collective_compute(
                    "AllGather", mybir.AluOpType.bypass,
                    replica_groups=[list(range(N_CORES))],
                    ins=[hslice[0:NC_N, :]],
                    outs=[tables[l][:, :]],
                )

                # ---- SpMM: AggT[0:H] = (A @ h)^T ----
                if PHASE < 3:
                    continue
                ci = 0
                gtiles = {}
                for sup in sups:
                    for s in range(S):
                        ntil = call_spec[ci]
                        gt = gpool.tile([128, TMAXCALL, H], f32, tag="g")
                        colbase = sum(cs * WIN // 16 for cs in call_spec[:ci])
                        nidx = ntil * WIN
                        idxt = stgpool.tile([128, TMAXCALL * 8], dt.int16,
                                            tag="idx")
                        nc.sync.dma_start(
                            out=idxt[:, :nidx // 16],
                            in_=idx_in[:, colbase:colbase + nidx // 16])
                        nc.gpsimd.dma_gather(
                            gt[:, 0:ntil, :],
                            tables[l][s * VS:(s + 1) * VS, :],
                            idxt[:, :nidx // 16],
                            nidx, nidx, H, single_packet=False)
                        gtiles[ci] = gt
                        ci += 1
                    for w in sup:
                        wts = win_tiles[w]
                        pw = ps_w.tile([H, WIN], f32, tag="pw")
                        for k, (cidx, colk, tglob) in enumerate(wts):
                            st = stpool.tile([128, 128], f32, tag="st")
                            nc.vector.tensor_scalar(
                                out=st[:], in0=iota[:],
                                scalar1=recl[:, tglob:tglob + 1],
                                scalar2=None, op0=mybir.AluOpType.is_equal)
                            nc.tensor.matmul(
                                pw[:], gtiles[cidx][:, colk, :], st[:],
                                start=(k == 0), stop=(k == len(wts) - 1))
                        nc.scalar.copy(
                            out=AggT[0:H, w * WIN:(w + 1) * WIN], in_=pw[:])

                # ---- dense: update + residual relu ----
                if PHASE < 4:
                    continue
                col = 0
                for cw in nchunks:
                    degt = dpool_sb.tile([H, DENSE_CHUNK], f32, tag="degt")
                    nc.sync.dma_start(out=degt[:, :cw],
                                      in_=degt_in[:, col:col + cw])
                    hd = dpool_sb.tile([H, DENSE_CHUNK], f32, tag="hd")
                    nc.vector.tensor_tensor(
                        out=hd[:, :cw], in0=Ht[0:H, col:col + cw],
                        in1=degt[:, :cw], op=mybir.AluOpType.mult)
                    ps1 = ps_mm.tile([H, DENSE_CHUNK], f32, tag="mm")
                    nc.tensor.matmul(ps1[:, :cw], w1e[:, l * H:(l + 1) * H],
                                     AggT[:, col:col + cw],
                                     start=True, stop=False)
                    nc.tensor.matmul(ps1[:, :cw], w2[:, l * H:(l + 1) * H],
                                     hd[:, :cw], start=False, stop=True)
                    agg2 = dpool_sb.tile([H, DENSE_CHUNK], f32, tag="agg2")
                    nc.scalar.copy(out=agg2[:, :cw], in_=ps1[:, :cw])
                    ps2 = ps_mm.tile([H, DENSE_CHUNK], f32, tag="mm")
                    nc.tensor.matmul(ps2[:, :cw], wu1e[:, l * H:(l + 1) * H],
                                     Ht[:, col:col + cw],
                                     start=True, stop=False)
                    nc.tensor.matmul(ps2[:, :cw], wu2[:, l * H:(l + 1) * H],
                                     agg2[:, :cw], start=False, stop=True)
                    rl = dpool_sb.tile([H, DENSE_CHUNK], f32, tag="rl")
                    nc.scalar.activation(rl[:, :cw], ps2[:, :cw],
                                         mybir.ActivationFunctionType.Relu)
                    nc.vector.tensor_tensor(
                        out=Ht[0:H, col:col + cw], in0=Ht[0:H, col:col + cw],
                        in1=rl[:, :cw], op=mybir.AluOpType.add)
                    col += cw

            # ---- pooling: pooled[g, f] = sum_n M[n, g] h[n, f] ----
            pp = ps_pool.tile([NG, H], f32, tag="pool")
            for w in range(W):
                tp = ps_tp.tile([128, H], f32, tag="tp")
                nc.tensor.transpose(out=tp[:], in_=Ht[0:H, w * WIN:(w + 1) * WIN],
                                    identity=ident[0:H, 0:H])
                hn = stgpool.tile([128, H], f32, tag="hn")
                nc.vector.tensor_copy(out=hn[:], in_=tp[:])
                mw = dpool_sb.tile([128, NG], f32, tag="mw")
                nc.sync.dma_start(out=mw[:], in_=mpool_in[w * WIN:(w + 1) * WIN, :])
                nc.tensor.matmul(pp[:], mw[:], hn[:],
                                 start=(w == 0), stop=(w == W - 1))
            pool_sb = cpool.tile([NG, H], f32)
            nc.vector.tensor_copy(out=pool_sb[:], in_=pp[:])
            nc.sync.dma_start(out=pool_out[:], in_=pool_sb[:])

    nc.compile()
    return nc


def kernel(x, edge_attr, edge_index, batch,
           W_embed, b_embed, W_edge, b_edge,
           msg_W, msg_b, upd_W, upd_b, W_pred, b_pred):
    x = np.asarray(x, np.float32)
    edge_attr = np.asarray(edge_attr, np.float32)
    edge_index = np.asarray(edge_index)
    batch = np.asarray(batch)
    send = np.asarray(edge_index[0], np.int64)
    rec = np.asarray(edge_index[1], np.int64)

    deg = np.bincount(rec, minlength=N).astype(np.float32)
    s_attr = np.bincount(rec, weights=edge_attr.astype(np.float64),
                         minlength=N).astype(np.float32)

    structure = _prep_structure(send, rec)

    # weights (shared across cores)
    msg_W = np.asarray(msg_W, np.float32)
    msg_b = np.asarray(msg_b, np.float32)
    upd_W = np.asarray(upd_W, np.float32)
    upd_b = np.asarray(upd_b, np.float32)
    W_edge = np.asarray(W_edge, np.float32)
    b_edge = np.asarray(b_edge, np.float32)
    w1e = np.zeros((H + 2, L * H), np.float32)
    w2 = np.zeros((H, L * H), np.float32)
    wu1e = np.zeros((H + 1, L * H), np.float32)
    wu2 = np.zeros((H, L * H), np.float32)
    for l in range(L):
        W1, W2m, W3 = msg_W[l, :H], msg_W[l, H:2 * H], msg_W[l, 2 * H:]
        u = W_edge @ W3                      # [1, H]
        c = b_edge @ W3 + msg_b[l]           # [H]
        w1e[:H, l * H:(l + 1) * H] = W1
        w1e[H, l * H:(l + 1) * H] = u[0]
        w1e[H + 1, l * H:(l + 1) * H] = c
        w2[:, l * H:(l + 1) * H] = W2m
        wu1e[:H, l * H:(l + 1) * H] = upd_W[l, :H]
        wu1e[H, l * H:(l + 1) * H] = upd_b[l]
        wu2[:, l * H:(l + 1) * H] = upd_W[l, H:]
    wemb = np.concatenate(
        [np.asarray(W_embed, np.float32), np.asarray(b_embed, np.float32)[None]], 0)

    in_maps = []
    for c in range(N_CORES):
        lo, hi = c * NC_N, (c + 1) * NC_N
        idx_sb, recl = _prep_core_arrays(structure, c)
        xT = np.zeros((FIN + 1, NPAD), np.float32)
        xT[:FIN, :NC_N] = x[lo:hi].T
        xT[FIN, :] = 1.0
        degt = np.zeros((H, NPAD), np.float32)
        degt[:, :NC_N] = deg[lo:hi][None, :]
        aggrows = np.zeros((2, NPAD), np.float32)
        aggrows[0, :NC_N] = s_attr[lo:hi]
        aggrows[1, :NC_N] = deg[lo:hi]
        mpool = np.zeros((NPAD, NG), np.float32)
        bl = batch[lo:hi].astype(np.int64)
        mpool[np.arange(NC_N), bl] = 1.0
        in_maps.append({
            "xT": xT, "degt": degt, "aggrows": aggrows, "mpool": mpool,
            "idx": idx_sb, "recl": recl,
            "w1e": w1e, "w2": w2, "wu1e": wu1e, "wu2": wu2, "wemb": wemb,
        })

    key = ("v1", PHASE, structure["TT"], tuple(structure["call_spec"]))
    if key not in _cache:
        _cache.clear()
        _cache[key] = _build_bass(structure)
    nc = _cache[key]

    trace = os.environ.get("K_TRACE", "0") == "1"
    res = bass_utils.run_bass_kernel_spmd(nc, in_maps, list(range(N_CORES)),
                                          trace=trace)
    kernel._last_results = res

    pooled = np.zeros((NG, H), np.float64)
    for c in range(N_CORES):
        pooled += res.results[c]["pool_out"].astype(np.float64)
    pred = pooled.astype(np.float32) @ np.asarray(W_pred, np.float32) \
        + np.asarray(b_pred, np.float32)
    return pred.squeeze(1)


# revision 11
# speedup vs baseline: 60.3088x; 60.3088x over previous
"""BasicMPNN Trainium2 kernel (8 NeuronCores, SPMD).

Math: since the message MLP is linear and applied before segment_sum,
    m = concat([h[send], h[rec], e], 1) @ msg_W + msg_b
    agg = segment_sum(m, rec)
        = (A @ h) @ W1 + (deg * h) @ W2 + s_attr x u_l + deg x c_l
  where A[r, s] = multiplicity of edge s->r, deg = A @ 1,
  s_attr[r] = sum of edge_attr into r, u_l = W_edge @ W3_l,
  c_l = b_edge @ W3_l + msg_b_l.
So each layer needs ONE sparse gather+reduce (A @ h) plus small dense
matmuls. Everything is computed feature-major on-chip ([64, nodes]).

Sharding: receiver-range partition, 12500 nodes / core. Full h lives
replicated in each core's DRAM (rebuilt per layer by AllGather). The
SpMM gathers h[send] rows with dma_gather (int16 idx -> 4 row-stripe
subtables of 25000 rows), then a one-hot matmul (S^T built on DVE by
comparing iota to the per-edge local receiver id) accumulates each
128-receiver window in PSUM, output directly feature-major.
"""

import os
import numpy as np

import concourse.bass as bass
import concourse.bacc as bacc
import concourse.tile as tile
from concourse import bass_utils, mybir, library_config
from concourse.masks import make_identity

dt = mybir.dt

# problem constants (hardcoded per contract)
N = 100000
E = 1600000
FIN = 16
H = 64
L = 3
NG = 64
N_CORES = 8
NC_N = N // N_CORES            # 12500 nodes per core
WIN = 128                      # receiver window
W = (NC_N + WIN - 1) // WIN    # 98 windows per core
NPAD = W * WIN                 # 12544
S = 4                          # send stripes (int16 idx limit)
VS = N // S                    # 25000 rows per stripe
SUPW = 4                       # windows per gather call group
DENSE_CHUNK = 512
PHASE = int(os.environ.get("K_PHASE", "4"))  # 1 embed+pool, 2 +AG, 3 +spmm, 4 full

_cache = {}


def _prep_structure(send, rec):
    """Host: bucket edges by (core, window, stripe); common (cross-core)
    tile counts; per-core idx / rec_local streams in global tile order."""
    core = rec // NC_N
    counts = np.zeros((N_CORES, W, S), np.int64)
    per_core = []
    for c in range(N_CORES):
        sel = core == c
        cs, cr = send[sel], rec[sel] - c * NC_N
        w = cr // WIN
        st = cs // VS
        key = (w * S + st).astype(np.int64)
        order = np.argsort(key, kind="stable")
        cs, cr, w, st, key = cs[order], cr[order], w[order], st[order], key[order]
        counts[c] = np.bincount(key, minlength=W * S).reshape(W, S)
        per_core.append((cs, cr, key))
    tiles = (np.max(counts, axis=0) + WIN - 1) // WIN  # [W, S] common
    # supergroups of SUPW windows
    sups = [list(range(i, min(i + SUPW, W))) for i in range(0, W, SUPW)]
    # global tile order: per sup, per stripe, per window
    tile_order = []          # list of (w, s)
    call_spec = []           # per (sup, s): (n_tiles,)
    win_tiles = [[] for _ in range(W)]   # w -> list of (call_idx, col_in_call)
    for sup in sups:
        for s in range(S):
            ntil = 0
            ci = len(call_spec)
            for w in sup:
                for _ in range(tiles[w, s]):
                    win_tiles[w].append((ci, ntil, len(tile_order)))
                    tile_order.append((w, s))
                    ntil += 1
            call_spec.append(ntil)
    TT = len(tile_order)
    return {
        "tiles": tiles, "sups": sups, "tile_order": tile_order,
        "call_spec": call_spec, "win_tiles": win_tiles, "TT": TT,
        "per_core": per_core,
    }


def _prep_core_arrays(structure, c):
    """Per-core idx (wrapped int16) and rec_local (f32) streams."""
    tiles = structure["tiles"]
    sups = structure["sups"]
    cs, cr, key = structure["per_core"][c]
    TT = structure["TT"]
    # bucket start offsets in the (w,s)-sorted edge array
    bc = np.bincount(key, minlength=W * S).reshape(W, S)
    starts = np.zeros(W * S + 1, np.int64)
    np.cumsum(bc.reshape(-1), out=starts[1:])
    idx_vals = np.zeros(TT * WIN, np.int16)
    rec_vals = np.full((TT * WIN,), -1.0, np.float32)
    t = 0
    for sup in sups:
        for s in range(S):
            for w in sup:
                k = w * S + s
                lo, n = starts[k], bc[w, s]
                nt = tiles[w, s]
                seg_idx = (cs[lo:lo + n] - s * VS).astype(np.int16)
                seg_rec = (cr[lo:lo + n] - w * WIN).astype(np.float32)
                idx_vals[t * WIN: t * WIN + n] = seg_idx
                rec_vals[t * WIN: t * WIN + n] = seg_rec
                t += nt
    assert t == TT
    # rec_local stream: [128, TT], column t = edges of tile t
    recl = rec_vals.reshape(TT, WIN).T.copy()
    # idx stream: per call, wrapped [16, n/16] replicated x8 -> concat cols
    blocks = []
    base = 0
    for ntil in structure["call_spec"]:
        n = ntil * WIN
        flat = idx_vals[base * WIN: base * WIN + n]
        wrapped = flat.reshape(n // 16, 16).T          # [16, n/16]
        blocks.append(np.tile(wrapped, (8, 1)))        # [128, n/16]
        base += ntil
    idx_sb = np.concatenate(blocks, axis=1)
    return idx_sb, recl


def _build_bass(structure):
    tiles = structure["tiles"]
    sups = structure["sups"]
    call_spec = structure["call_spec"]
    win_tiles = structure["win_tiles"]
    TT = structure["TT"]
    IDXCOLS = sum(ntil * WIN // 16 for ntil in call_spec)
    TMAXCALL = max(call_spec)

    nc = bacc.Bacc("TRN2", target_bir_lowering=False, debug=False,
                   num_devices=N_CORES)
    f32 = dt.float32

    # I/O
    xT_in = nc.dram_tensor("xT", [FIN + 1, NPAD], f32, kind="ExternalInput")
    degt_in = nc.dram_tensor("degt", [H, NPAD], f32, kind="ExternalInput")
    aggrows_in = nc.dram_tensor("aggrows", [2, NPAD], f32, kind="ExternalInput")
    mpool_in = nc.dram_tensor("mpool", [NPAD, NG], f32, kind="ExternalInput")
    idx_in = nc.dram_tensor("idx", [128, IDXCOLS], dt.int16, kind="ExternalInput")
    recl_in = nc.dram_tensor("recl", [128, TT], f32, kind="ExternalInput")
    w1e_in = nc.dram_tensor("w1e", [H + 2, L * H], f32, kind="ExternalInput")
    w2_in = nc.dram_tensor("w2", [H, L * H], f32, kind="ExternalInput")
    wu1e_in = nc.dram_tensor("wu1e", [H + 1, L * H], f32, kind="ExternalInput")
    wu2_in = nc.dram_tensor("wu2", [H, L * H], f32, kind="ExternalInput")
    wemb_in = nc.dram_tensor("wemb", [FIN + 1, H], f32, kind="ExternalInput")
    pool_out = nc.dram_tensor("pool_out", [NG, H], f32, kind="ExternalOutput")

    nchunks = [DENSE_CHUNK] * (NPAD // DENSE_CHUNK)
    if NPAD % DENSE_CHUNK:
        nchunks.append(NPAD % DENSE_CHUNK)

    with tile.TileContext(nc) as tc:
        with (
            tc.tile_pool(name="dram", bufs=1, space="DRAM") as dpool,
            tc.tile_pool(name="const", bufs=1) as cpool,
            tc.tile_pool(name="gpool", bufs=6) as gpool,
            tc.tile_pool(name="stp", bufs=4) as stpool,
            tc.tile_pool(name="dense", bufs=2) as dpool_sb,
            tc.tile_pool(name="stg", bufs=3) as stgpool,
            tc.tile_pool(name="ps_w", bufs=2, space="PSUM") as ps_w,
            tc.tile_pool(name="ps_tp", bufs=2, space="PSUM") as ps_tp,
            tc.tile_pool(name="ps_mm", bufs=3, space="PSUM") as ps_mm,
            tc.tile_pool(name="ps_pool", bufs=1, space="PSUM") as ps_pool,
        ):
            tables = [dpool.tile([N, H], f32, addr_space="Shared",
                                 name=f"table{l}") for l in range(L)]
            hslice = dpool.tile([NPAD, H], f32)

            nc.gpsimd.load_library(library_config.mlp)

            # ---- constants ----
            iota = cpool.tile([128, 128], f32)
            nc.gpsimd.iota(iota[:], pattern=[[1, 128]], base=0,
                           channel_multiplier=0,
                           allow_small_or_imprecise_dtypes=True)
            ident = cpool.tile([128, 128], f32)
            make_identity(nc, ident[:])
            recl = cpool.tile([128, TT], f32)
            nc.sync.dma_start(out=recl[:], in_=recl_in[:])
            w1e = cpool.tile([H + 2, L * H], f32)
            nc.sync.dma_start(out=w1e[:], in_=w1e_in[:])
            w2 = cpool.tile([H, L * H], f32)
            nc.sync.dma_start(out=w2[:], in_=w2_in[:])
            wu1e = cpool.tile([H + 1, L * H], f32)
            nc.sync.dma_start(out=wu1e[:], in_=wu1e_in[:])
            wu2 = cpool.tile([H, L * H], f32)
            nc.sync.dma_start(out=wu2[:], in_=wu2_in[:])
            wemb = cpool.tile([FIN + 1, H], f32)
            nc.sync.dma_start(out=wemb[:], in_=wemb_in[:])

            Ht = cpool.tile([H + 1, NPAD], f32)       # row H = ones
            nc.vector.memset(Ht[H:H + 1, :], 1.0)
            AggT = cpool.tile([H + 2, NPAD], f32)     # rows H,H+1 = s_attr,deg
            nc.sync.dma_start(out=AggT[H:H + 2, :], in_=aggrows_in[:])

            # ---- embed: Ht = (x @ W_embed + b_embed)^T ----
            col = 0
            for cw in nchunks:
                xt = dpool_sb.tile([FIN + 1, DENSE_CHUNK], f32, tag="xt")
                nc.sync.dma_start(out=xt[:, :cw], in_=xT_in[:, col:col + cw])
                ps = ps_mm.tile([H, DENSE_CHUNK], f32, tag="mm")
                nc.tensor.matmul(ps[:, :cw], wemb[:], xt[:, :cw],
                                 start=True, stop=True)
                nc.scalar.copy(out=Ht[0:H, col:col + cw], in_=ps[:, :cw])
                col += cw

            for l in range(L if PHASE >= 2 else 0):
                # ---- write back h -> hslice -> AllGather -> table ----
                for gi, sup in enumerate(sups):
                    stg = stgpool.tile([128, SUPW * H], f32, tag="stg")
                    for j, w in enumerate(sup):
                        tp = ps_tp.tile([128, H], f32, tag="tp")
                        nc.tensor.transpose(
                            out=tp[:], in_=Ht[0:H, w * WIN:(w + 1) * WIN],
                            identity=ident[0:H, 0:H])
                        nc.vector.tensor_copy(
                            out=stg[:, j * H:(j + 1) * H], in_=tp[:])
                    nw = len(sup)
                    dst = hslice[:].rearrange("(w p) f -> p w f", p=WIN)
                    nc.sync.dma_start(
                        out=dst[:, sup[0]:sup[0] + nw, :],
                        in_=stg[:, :nw * H].rearrange("p (w f) -> p w f", f=H))
                nc.gpsimd.collective_compute(
                    "AllGather", mybir.AluOpType.bypass,
                    replica_groups=[list(range(N_CORES))],
                    ins=[hslice[0:NC_N, :]],
                    outs=[tables[l][:, :]],
                )

                # ---- SpMM: AggT[0:H] = (A @ h)^T ----
                if PHASE < 3:
                    continue
                ci = 0
                gtiles = {}
                for sup in sups:
                    for s in range(S):
                        ntil = call_spec[ci]
                        gt = gpool.tile([128, TMAXCALL, H], f32, tag="g")
                        colbase = sum(cs * WIN // 16 for cs in call_spec[:ci])
                        nidx = ntil * WIN
                        idxt = stgpool.tile([128, TMAXCALL * 8], dt.int16,
                                            tag="idx")
                        nc.sync.dma_start(
                            out=idxt[:, :nidx // 16],
                            in_=idx_in[:, colbase:colbase + nidx // 16])
                        nc.gpsimd.dma_gather(
                            gt[:, 0:ntil, :],
                            tables[l][s * VS:(s + 1) * VS, :],
                            idxt[:, :nidx // 16],
                            nidx, nidx, H, single_packet=False)
                        gtiles[ci] = gt
                        ci += 1
                    for w in sup:
                        wts = win_tiles[w]
                        pw = ps_w.tile([H, WIN], f32, tag="pw")
                        for k, (cidx, colk, tglob) in enumerate(wts):
                            st = stpool.tile([128, 128], f32, tag="st")
                            nc.vector.tensor_scalar(
                                out=st[:], in0=iota[:],
                                scalar1=recl[:, tglob:tglob + 1],
                                scalar2=None, op0=mybir.AluOpType.is_equal)
                            nc.tensor.matmul(
                                pw[:], gtiles[cidx][:, colk, :], st[:],
                                start=(k == 0), stop=(k == len(wts) - 1))
                        nc.scalar.copy(
                            out=AggT[0:H, w * WIN:(w + 1) * WIN], in_=pw[:])

                # ---- dense: update + residual relu ----
                if PHASE < 4:
                    continue
                col = 0
                for cw in nchunks:
                    degt = dpool_sb.tile([H, DENSE_CHUNK], f32, tag="degt")
                    nc.sync.dma_start(out=degt[:, :cw],
                                      in_=degt_in[:, col:col + cw])
                    hd = dpool_sb.tile([H, DENSE_CHUNK], f32, tag="hd")
                    nc.vector.tensor_tensor(
                        out=hd[:, :cw], in0=Ht[0:H, col:col + cw],
                        in1=degt[:, :cw], op=mybir.AluOpType.mult)
                    ps1 = ps_mm.tile([H, DENSE_CHUNK], f32, tag="mm")
                    nc.tensor.matmul(ps1[:, :cw], w1e[:, l * H:(l + 1) * H],
                                     AggT[:, col:col + cw],
                                     start=True, stop=False)
                    nc.tensor.matmul(ps1[:, :cw], w2[:, l * H:(l + 1) * H],
                                     hd[:, :cw], start=False, stop=True)
                    agg2 = dpool_sb.tile([H, DENSE_CHUNK], f32, tag="agg2")
                    nc.scalar.copy(out=agg2[:, :cw], in_=ps1[:, :cw])
                    ps2 = ps_mm.tile([H, DENSE_CHUNK], f32, tag="mm")
                    nc.tensor.matmul(ps2[:, :cw], wu1e[:, l * H:(l + 1) * H],
                                     Ht[:, col:col + cw],
                                     start=True, stop=False)
                    nc.tensor.matmul(ps2[:, :cw], wu2[:, l * H:(l + 1) * H],
                                     agg2[:, :cw], start=False, stop=True)
                    rl = dpool_sb.tile([H, DENSE_CHUNK], f32, tag="rl")
                    nc.scalar.activation(rl[:, :cw], ps2[:, :cw],
                                         mybir.ActivationFunctionType.Relu)
                    nc.vector.tensor_tensor(
                        out=Ht[0:H, col:col + cw], in0=Ht[0:H, col:col + cw],
                        in1=rl[:, :cw], op=mybir.AluOpType.add)
                    col += cw

            # ---- pooling: pooled[g, f] = sum_n M[n, g] h[n, f] ----
            pp = ps_pool.tile([NG, H], f32, tag="pool")
            for w in range(W):
                tp = ps_tp.tile([128, H], f32, tag="tp")
                nc.tensor.transpose(out=tp[:], in_=Ht[0:H, w * WIN:(w + 1) * WIN],
                                    identity=ident[0:H, 0:H])
                hn = stgpool.tile([128, H], f32, tag="hn")
                nc.vector.tensor_copy(out=hn[:], in_=tp[:])
                mw = dpool_sb.tile([128, NG], f32, tag="mw")
                nc.sync.dma_start(out=mw[:], in_=mpool_in[w * WIN:(w + 1) * WIN, :])
                nc.tensor.matmul(pp[:], mw[:], hn[:],
                                 start=(w == 0), stop=(w == W - 1))
            pool_sb = cpool.tile([NG, H], f32)
            nc.vector.tensor_copy(out=pool_sb[:], in_=pp[:])
            nc.sync.dma_start(out=pool_out[:], in_=pool_sb[:])

    nc.compile()
    return nc


def kernel(x, edge_attr, edge_index, batch,
           W_embed, b_embed, W_edge, b_edge,
           msg_W, msg_b, upd_W, upd_b, W_pred, b_pred):
    x = np.asarray(x, np.float32)
    edge_attr = np.asarray(edge_attr, np.float32)
    edge_index = np.asarray(edge_index)
    batch = np.asarray(batch)
    send = np.asarray(edge_index[0], np.int64)
    rec = np.asarray(edge_index[1], np.int64)

    deg = np.bincount(rec, minlength=N).astype(np.float32)
    s_attr = np.bincount(rec, weights=edge_attr.astype(np.float64),
                         minlength=N).astype(np.float32)

    structure = _prep_structure(send, rec)

    # weights (shared across cores)
    msg_W = np.asarray(msg_W, np.float32)
    msg_b = np.asarray(msg_b, np.float32)
    upd_W = np.asarray(upd_W, np.float32)
    upd_b = np.asarray(upd_b, np.float32)
    W_edge = np.asarray(W_edge, np.float32)
    b_edge = np.asarray(b_edge, np.float32)
    w1e = np.zeros((H + 2, L * H), np.float32)
    w2 = np.zeros((H, L * H), np.float32)
    wu1e = np.zeros((H + 1, L * H), np.float32)
    wu2 = np.zeros((H, L * H), np.float32)
    for l in range(L):
        W1, W2m, W3 = msg_W[l, :H], msg_W[l, H:2 * H], msg_W[l, 2 * H:]
        u = W_edge @ W3                      # [1, H]
        c = b_edge @ W3 + msg_b[l]           # [H]
        w1e[:H, l * H:(l + 1) * H] = W1
        w1e[H, l * H:(l + 1) * H] = u[0]
        w1e[H + 1, l * H:(l + 1) * H] = c
        w2[:, l * H:(l + 1) * H] = W2m
        wu1e[:H, l * H:(l + 1) * H] = upd_W[l, :H]
        wu1e[H, l * H:(l + 1) * H] = upd_b[l]
        wu2[:, l * H:(l + 1) * H] = upd_W[l, H:]
    wemb = np.concatenate(
        [np.asarray(W_embed, np.float32), np.asarray(b_embed, np.float32)[None]], 0)

    in_maps = []
    for c in range(N_CORES):
        lo, hi = c * NC_N, (c + 1) * NC_N
        idx_sb, recl = _prep_core_arrays(structure, c)
        xT = np.zeros((FIN + 1, NPAD), np.float32)
        xT[:FIN, :NC_N] = x[lo:hi].T
        xT[FIN, :] = 1.0
        degt = np.zeros((H, NPAD), np.float32)
        degt[:, :NC_N] = deg[lo:hi][None, :]
        aggrows = np.zeros((2, NPAD), np.float32)
        aggrows[0, :NC_N] = s_attr[lo:hi]
        aggrows[1, :NC_N] = deg[lo:hi]
        mpool = np.zeros((NPAD, NG), np.float32)
        bl = batch[lo:hi].astype(np.int64)
        mpool[np.arange(NC_N), bl] = 1.0
        in_maps.append({
            "xT": xT, "degt": degt, "aggrows": aggrows, "mpool": mpool,
            "idx": idx_sb, "recl": recl,
            "w1e": w1e, "w2": w2, "wu1e": wu1e, "wu2": wu2, "wemb": wemb,
        })

    key = ("v1", PHASE, structure["TT"], tuple(structure["call_spec"]))
    if key not in _cache:
        _cache.clear()
        _cache[key] = _build_bass(structure)
    nc = _cache[key]

    trace = os.environ.get("K_TRACE", "0") == "1"
    res = bass_utils.run_bass_kernel_spmd(nc, in_maps, list(range(N_CORES)),
                                          trace=trace)
    kernel._last_results = res
    kernel._last_in_maps = in_maps

    pooled = np.zeros((NG, H), np.float64)
    for c in range(N_CORES):
        pooled += res.results[c]["pool_out"].astype(np.float64)
    pred = pooled.astype(np.float32) @ np.asarray(W_pred, np.float32) \
        + np.asarray(b_pred, np.float32)
    return pred.squeeze(1)
